# revision 2
# baseline (speedup 1.0000x reference)
"""Trainium2 Bass kernel for a dense transformer encoder layer (v2).

Problem (hardcoded): x [2, 2048, 1024], 16 heads, FFN 4096, fp32 I/O,
post-LN residual blocks, additive mask before softmax (graded mask is
all-ones -> masking compiled out).

Sharding: sequence-parallel, 512 queries per core (cores 0-3 batch 0,
cores 4-7 batch 1). Each core computes full-batch K/V itself (no
collectives on this stack) and keeps K^T / V' resident in SBUF in bf16
(no DRAM bounce). All matmul operands are bf16 (fp32 PSUM accumulation);
LayerNorm statistics and residual adds run in fp32.

Phase structure (emission order == engine-queue order):
  AB: per m-block (2 heads): K^T m -> Q^T m -> interleaved
      [scores(h)/exp(h) on ACT | attnV(h-1) 65-row matmuls] -> V blocks.
      exp (the only heavy ACT work) hides under PE; attnV accumulates
      o in natural [q, dh] orientation with a ones-column denominator.
  C:  o^T via PE transposes, output projection, +res +bias, LN1.
  D:  FFN1 (relu on ACT), FFN2, +res +bias, LN2, store.

Matmul layouts (out = lhsT.T @ rhs, contraction on partition dim):
  Q^T/K^T : lhsT = W tile [din,dout], rhs = x^T [din,tok]   -> [dout,tok]
  V       : lhsT = x^T [din,tok], rhs = Wv [din,dout]       -> [tok,dout]
  scoresT : lhsT = K^T head [dh,kpos], rhs = Q^T [dh,q]     -> [kpos,q]
  attnV   : lhsT = expT [kpos,qtile], rhs = V'|1 [kpos,65]  -> [q,dh+1]
  outproj : lhsT = o^T [din,qtile], rhs = Wp [din,dout]     -> [q,dout]
  FFN1    : lhsT = W1 tile [din,dffn], rhs = xln1^T [din,q] -> [dffn,q]
  FFN2    : lhsT = h^T [dffn,qtile], rhs = W2 [dffn,dout]   -> [q,dout]
"""

import numpy as np

import concourse.bass as bass
import concourse.mybir as mybir
import concourse.tile as tile
from concourse.bass_utils import run_bass_kernel_spmd
from concourse.masks import make_identity
from concourse.vector_clock import ScopedClock

FP32 = mybir.dt.float32
BF16 = mybir.dt.bfloat16
AF = mybir.ActivationFunctionType
ALU = mybir.AluOpType

P = 128
D = 1024
F = 4096
H = 16
DH = 64
S = 2048          # tokens per batch
TPC = 512         # queries per core
NB = D // P       # 8 dout blocks (= head pairs)
KB = D // P       # 8 contraction tiles over D
FB = F // P       # 32 dffn tiles
QT = TPC // P     # 4 query tiles
KT16 = S // P     # 16 kpos tiles
NG = S // TPC     # 4 kpos 512-slices
VW = H * (DH + 1)  # 1040: V' row width (ones column per head)
SCALE = DH ** -0.5
EPS = 1e-6
N_CORES = 8


# --- Tile tail-drain fix: this walrus build allows only one sem-wait per
# instruction; Tile's final drain accumulates several. Split them across
# dedicated nops before draining.
def _patched_drain_and_barrier(self, tick_clock, wait_clock):
    probe = self.nc.sync.nop(nofuse=True, hint="drain_wait_split")
    wait_clock.add_sem_waits(probe.ins, ScopedClock({None: tick_clock.global_clock}))
    si = probe.ins.sync_info
    if si is not None and si.on_wait and len(si.on_wait) > 1:
        waits = list(si.on_wait)
        si.on_wait = waits[:1]
        for w in waits[1:]:
            extra = self.nc.sync.nop(nofuse=True, hint="drain_wait_split")
            esi = extra.ins.sync_info
            if esi is None:
                extra.ins.sync_info = mybir.SyncInfo(on_wait=[w], on_update=[])
            else:
                esi.on_wait = [w]
    self.nc.sync.drain()
    self.nc.all_engine_barrier()
    assert self.sems is not None
    popped = self.nc._tile_sem_poison_stack.pop()
    assert popped is self._sem_poison
    self.nc.clear_and_free_semaphores(list(self.sems.allocated().values()))
    self.nc.all_engine_barrier()


if getattr(tile.TileContext, "_drain_patch", None) is None:
    tile.TileContext._drain_and_barrier = _patched_drain_and_barrier
    tile.TileContext._drain_patch = True


def _split_waits(nc):
    """Walrus codegen accepts at most one sem-wait per instruction (two on
    EventSemaphore). Tile's scheduler can emit more; hoist the surplus onto
    same-engine EventSemaphore instructions inserted just before."""
    uid = [0]
    for bb in nc.m.functions[0].blocks:
        new_insts = []
        for inst in bb.instructions:
            si = inst.sync_info
            limit = 2 if isinstance(inst, mybir.InstEventSemaphore) else 1
            if si is not None and si.on_wait and len(si.on_wait) > limit:
                waits = list(si.on_wait)
                extra, keep = waits[:-limit], waits[-limit:]
                for i in range(0, len(extra), 2):
                    uid[0] += 1
                    ev = mybir.InstEventSemaphore(
                        name=f"I-wsplit-{uid[0]}",
                        engine=inst.engine,
                        sync_info=mybir.SyncInfo(
                            on_wait=extra[i:i + 2], on_update=[]),
                    )
                    nc.register_instruction(ev)
                    new_insts.append(ev)
                si.on_wait = keep
            new_insts.append(inst)
        if len(new_insts) != len(bb.instructions):
            bb.instructions[:] = new_insts


def _ln_chain(nc, pool, y, out_ap, gamma_b, beta_b, eps_t, beta_eng=None):
    """LayerNorm over the free dim of y [128, D] (torch semantics:
    unbiased std, denominator std + eps), writing to out_ap (fp32)."""
    s1 = pool.tile([P, 1], FP32, tag="ln_s1")
    nc.vector.reduce_sum(s1[:], y[:], axis=mybir.AxisListType.X)
    mn = pool.tile([P, 1], FP32, tag="ln_mn")
    nc.scalar.mul(mn[:], s1[:], 1.0 / D)
    cen = pool.tile([P, D], FP32, tag="ln_cen")
    nc.vector.tensor_scalar_sub(cen[:], y[:], mn[:])
    sq = pool.tile([P, D], FP32, tag="ln_sq")
    ss = pool.tile([P, 1], FP32, tag="ln_ss")
    nc.scalar.activation(sq[:], cen[:], AF.Square, accum_out=ss[:])
    var = pool.tile([P, 1], FP32, tag="ln_var")
    nc.scalar.mul(var[:], ss[:], 1.0 / (D - 1))
    std = pool.tile([P, 1], FP32, tag="ln_std")
    nc.scalar.activation(std[:], var[:], AF.Sqrt)
    nc.scalar.activation(std[:], std[:], AF.Identity, bias=eps_t[:])
    inv = pool.tile([P, 1], FP32, tag="ln_inv")
    nc.vector.reciprocal(inv[:], std[:])
    nc.vector.scalar_tensor_tensor(
        out_ap, cen[:], inv[:], gamma_b[:], op0=ALU.mult, op1=ALU.mult
    )
    (beta_eng or nc.vector).tensor_add(out_ap, out_ap, beta_b[:])


def build_program(use_mask: bool) -> bass.Bass:
    nc = bass.Bass(target_bir_lowering=False, debug=False)

    # ---- I/O ----
    xT_d = nc.dram_tensor("xT", [D, S], BF16, kind="ExternalInput")
    xTq_d = nc.dram_tensor("xTq", [D, TPC], BF16, kind="ExternalInput")
    xblk_d = nc.dram_tensor("xblk", [TPC, D], BF16, kind="ExternalInput")
    wqt_d = nc.dram_tensor("wqt", [D, D], BF16, kind="ExternalInput")
    wkt_d = nc.dram_tensor("wkt", [D, D], BF16, kind="ExternalInput")
    wv_d = nc.dram_tensor("wv", [D, D], BF16, kind="ExternalInput")
    wp_d = nc.dram_tensor("wp", [D, D], BF16, kind="ExternalInput")
    w1t_d = nc.dram_tensor("w1t", [F, D], BF16, kind="ExternalInput")
    w2_d = nc.dram_tensor("w2", [F, D], BF16, kind="ExternalInput")
    bq_d = nc.dram_tensor("bq", [D], FP32, kind="ExternalInput")
    bk_d = nc.dram_tensor("bk", [D], FP32, kind="ExternalInput")
    bv_d = nc.dram_tensor("bv", [D], BF16, kind="ExternalInput")
    b1_d = nc.dram_tensor("b1", [F], FP32, kind="ExternalInput")
    b2_d = nc.dram_tensor("b2", [D], FP32, kind="ExternalInput")
    g1_d = nc.dram_tensor("g1", [D], FP32, kind="ExternalInput")
    be1_d = nc.dram_tensor("be1", [D], FP32, kind="ExternalInput")
    g2_d = nc.dram_tensor("g2", [D], FP32, kind="ExternalInput")
    be2_d = nc.dram_tensor("be2", [D], FP32, kind="ExternalInput")
    if use_mask:
        maskT_d = nc.dram_tensor("maskT", [S, TPC], BF16, kind="ExternalInput")
    out_d = nc.dram_tensor("out", [TPC, D], FP32, kind="ExternalOutput")

    with tile.TileContext(nc) as tc:
        _build_body(
            nc, tc, use_mask,
            xT_d, xTq_d, xblk_d, wqt_d, wkt_d, wv_d, wp_d, w1t_d, w2_d,
            bq_d, bk_d, bv_d, b1_d, b2_d, g1_d, be1_d, g2_d, be2_d,
            maskT_d if use_mask else None, out_d,
        )
    _split_waits(nc)
    return nc


def _build_body(nc, tc, use_mask, xT_d, xTq_d, xblk_d, wqt_d, wkt_d, wv_d,
                wp_d, w1t_d, w2_d, bq_d, bk_d, bv_d, b1_d, b2_d,
                g1_d, be1_d, g2_d, be2_d, maskT_d, out_d):
    from contextlib import ExitStack

    with ExitStack() as top:
        consts = top.enter_context(tc.tile_pool(name="consts", bufs=1))
        ident = consts.tile([P, P], BF16)
        make_identity(nc, ident[:])
        bq_c = consts.tile([P, NB], FP32)
        bk_c = consts.tile([P, NB], FP32)
        bv_b = consts.tile([P, D], BF16)
        b1_c = consts.tile([P, FB], FP32)
        eps_t = consts.tile([P, 1], FP32)
        nc.vector.memset(eps_t[:], EPS)

        # o^T + Wp live from phase AB through phase C
        opp_stack = ExitStack()
        opp = opp_stack.enter_context(tc.tile_pool(name="opp", bufs=1, side="right"))
        oT = opp.tile([P, KB * TPC], BF16)          # o^T for outproj, 8KB
        wp_sb = opp.tile([P, KB * D], BF16)         # full Wp, 16KB
        o_nat = opp.tile([P, QT * D], BF16)         # attn out, natural, 8KB
        xblk_sb = opp.tile([P, QT * D], BF16)       # residual x (+bp), 8KB

        # Attention persistents (die at end of phase AB)
        abp_stack = ExitStack()
        abp = abp_stack.enter_context(tc.tile_pool(name="abp", bufs=1))
        KT = abp.tile([P, NB * S], BF16)            # K^T, 32KB/part
        QTs = abp.tile([P, NB * TPC], BF16)         # Q^T, 8KB/part
        VP = abp.tile([P, KT16 * VW], BF16)         # V' with ones col, 33KB

        # ones columns of V'
        vp4 = VP[:].rearrange("p (t h j) -> p t h j", h=H, j=DH + 1)
        nc.vector.memset(vp4[:, :, :, DH], 1.0)

        # ================= Phase AB: QKV + attention =================
        # Phase A: V (+K0/Q0) fully emitted first -- attnV consumes every V'
        # tile, and emission order defines dataflow. Phase B: per head-pair,
        # K/Q projection, then scores+exp (kt-pair-wide PSUM -> one exp per
        # 1024 cols) interleaved with the previous head's attnV drain.
        with ExitStack() as ab_stack:
            xp = ab_stack.enter_context(tc.tile_pool(name="ab_x", bufs=1))
            wpool = ab_stack.enter_context(tc.tile_pool(name="ab_w", bufs=2))
            scr = ab_stack.enter_context(tc.tile_pool(name="ab_scr", bufs=4))
            kqv_ps = ab_stack.enter_context(
                tc.tile_pool(name="ab_kq_ps", bufs=2, space="PSUM"))
            xt = xp.tile([P, KB * S], BF16)          # x^T full batch, 32KB
            xtq = xp.tile([P, KB * TPC], BF16)       # own x^T cols, 8KB
            if use_mask:
                mk_sb = xp.tile([P, KT16 * TPC], BF16)   # additive maskT, 16KB

            # Weight staging tiles, DMA-prefetched ahead.
            wk_t, wq_t = {}, {}

            def prefetch_w(m):
                wk_t[m] = wpool.tile([P, KB * P], BF16, tag="wkm",
                                     name=f"wk_{m}")
                nc.sync.dma_start(wk_t[m][:], wkt_d.ap()[m * P:(m + 1) * P, :])
                wq_t[m] = wpool.tile([P, KB * P], BF16, tag="wqm",
                                     name=f"wq_{m}")
                nc.sync.dma_start(wq_t[m][:], wqt_d.ap()[m * P:(m + 1) * P, :])

            def x_ng(ng):
                nc.sync.dma_start(
                    xt[:].rearrange("p (k c) -> p k c", k=KB)[:, :, ng * TPC:
                                                             (ng + 1) * TPC],
                    xT_d.ap().rearrange("(k p) c -> p k c", p=P)[:, :,
                                                                ng * TPC:
                                                                (ng + 1) * TPC])

            def emit_K(m):
                wk_m = wk_t.pop(m)
                for ng in range(NG):
                    ps = kqv_ps.tile([P, TPC], FP32, tag="kqvps")
                    for k in range(KB):
                        nc.tensor.matmul(
                            ps[:],
                            lhsT=wk_m[:, k * P:(k + 1) * P],
                            rhs=xt[:, k * S + ng * TPC: k * S + (ng + 1) * TPC],
                            start=(k == 0), stop=(k == KB - 1),
                        )
                    nc.vector.tensor_scalar_add(
                        KT[:, m * S + ng * TPC: m * S + (ng + 1) * TPC],
                        ps[:], bk_c[:, m:m + 1])

            def emit_Q(m):
                wq_m = wq_t.pop(m)
                ps = kqv_ps.tile([P, TPC], FP32, tag="kqvps")
                for k in range(KB):
                    nc.tensor.matmul(
                        ps[:],
                        lhsT=wq_m[:, k * P:(k + 1) * P],
                        rhs=xtq[:, k * TPC:(k + 1) * TPC],
                        start=(k == 0), stop=(k == KB - 1),
                    )
                nc.vector.tensor_scalar_add(
                    QTs[:, m * TPC:(m + 1) * TPC], ps[:], bq_c[:, m:m + 1])

            # ---------- Phase A ----------
            with (
                tc.tile_pool(name="a_wv", bufs=1) as wvp,
                tc.tile_pool(name="a_v_ps", bufs=3, space="PSUM") as v_ps,
            ):
                wv_sb = wvp.tile([P, KB * D], BF16)      # full Wv, 16KB

                def emit_V(mt):
                    for nd in range(2):
                        ps = v_ps.tile([P, TPC], FP32, tag="vps")
                        for k in range(KB):
                            nc.tensor.matmul(
                                ps[:],
                                lhsT=xt[:, k * S + mt * P: k * S + (mt + 1) * P],
                                rhs=wv_sb[:, k * D + nd * TPC:
                                          k * D + (nd + 1) * TPC],
                                start=(k == 0), stop=(k == KB - 1),
                            )
                        dst = VP[:, mt * VW:(mt + 1) * VW].rearrange(
                            "p (h j) -> p h j", j=DH + 1)[:, nd * 8:(nd + 1) * 8,
                                                          0:DH]
                        nc.vector.tensor_copy(
                            dst, ps[:].rearrange("p (h j) -> p h j", j=DH))

                x_ng(0)
                prefetch_w(0)
                nc.sync.dma_start(
                    xtq[:].rearrange("p (k c) -> p k c", k=KB),
                    xTq_d.ap().rearrange("(k p) c -> p k c", p=P))
                nc.sync.dma_start(
                    wv_sb[:].rearrange("p (k c) -> p k c", k=KB),
                    wv_d.ap().rearrange("(k p) c -> p k c", p=P))
                nc.sync.dma_start(bq_c[:],
                                  bq_d.ap().rearrange("(b p) -> p b", p=P))
                nc.sync.dma_start(bk_c[:],
                                  bk_d.ap().rearrange("(b p) -> p b", p=P))
                nc.sync.dma_start(bv_b[:],
                                  bv_d.ap()[None, :].to_broadcast((P, D)))
                x_ng(1)
                prefetch_w(1)
                x_ng(2)
                x_ng(3)
                if use_mask:
                    for kt in range(KT16):
                        nc.sync.dma_start(
                            mk_sb[:, kt * TPC:(kt + 1) * TPC],
                            maskT_d.ap()[kt * P:(kt + 1) * P, :])
                emit_K(0)
                emit_Q(0)
                for mt in range(KT16):
                    emit_V(mt)
                    if mt == 3:
                        prefetch_w(2)

            # ---------- Phase B ----------
            epool = ab_stack.enter_context(
                tc.tile_pool(name="ab_exp", bufs=(8 if use_mask else 16)))
            sc_ps = ab_stack.enter_context(
                tc.tile_pool(name="ab_sc_ps", bufs=2, space="PSUM"))
            acc_ps = ab_stack.enter_context(
                tc.tile_pool(name="ab_acc_ps", bufs=2, space="PSUM"))
            KTP = KT16 // 2   # scores PSUM tiles span a kt pair (2 banks)
            exp_tiles = {}    # (h, ktp) -> [128, 2*TPC] bf16 exp tile

            def emit_scores_exp(h, ktp):
                m, hpar = divmod(h, 2)
                hp = hpar * DH
                sp = sc_ps.tile([P, 2 * TPC], FP32, tag="spsum")
                for j in range(2):
                    kt = 2 * ktp + j
                    nc.tensor.matmul(
                        sp[:, j * TPC:(j + 1) * TPC],
                        lhsT=KT[hp:hp + DH, m * S + kt * P: m * S + (kt + 1) * P],
                        rhs=QTs[hp:hp + DH, m * TPC:(m + 1) * TPC],
                        start=True, stop=True,
                    )
                if use_mask:
                    for j in range(2):
                        kt = 2 * ktp + j
                        nc.vector.tensor_add(
                            sp[:, j * TPC:(j + 1) * TPC],
                            sp[:, j * TPC:(j + 1) * TPC],
                            mk_sb[:, kt * TPC:(kt + 1) * TPC])
                et = epool.tile([P, 2 * TPC], BF16, tag="expT")
                nc.scalar.activation(et[:], sp[:], AF.Exp, scale=SCALE)
                exp_tiles[(h, ktp)] = et

            acc_of = {}             # h -> acc tile while accumulating

            def emit_attnV_slot(h, slot):
                """4 attnV matmuls: flattened (qt-major, kt-minor) chunk
                `slot` of head h. Groups are qt-sequential (one pending
                accumulation group per PSUM bank at a time)."""
                if slot == 0:
                    acc_of[h] = acc_ps.tile([P, QT * (DH + 1)], FP32,
                                            tag="accps", name=f"acc_{h}")
                acc = acc_of[h]
                for i in range(4 * slot, 4 * slot + 4):
                    qt, kt = divmod(i, KT16)
                    ktp, j = divmod(kt, 2)
                    et = exp_tiles[(h, ktp)]
                    nc.tensor.matmul(
                        acc[:, qt * (DH + 1):(qt + 1) * (DH + 1)],
                        lhsT=et[:, j * TPC + qt * P: j * TPC + (qt + 1) * P],
                        rhs=VP[:, kt * VW + h * (DH + 1):
                               kt * VW + (h + 1) * (DH + 1)],
                        start=(kt == 0), stop=(kt == KT16 - 1),
                    )
                if slot == KT16 - 1:
                    for kk in range(KTP):
                        del exp_tiles[(h, kk)]

            def emit_norm(h):
                acc = acc_of.pop(h)
                for qt in range(QT):
                    rr = scr.tile([P, 1], FP32, tag="rrow")
                    nc.vector.reciprocal(
                        rr[:], acc[:, qt * (DH + 1) + DH: qt * (DH + 1) + DH + 1])
                    nc.vector.scalar_tensor_tensor(
                        o_nat[:, qt * D + h * DH: qt * D + (h + 1) * DH],
                        acc[:, qt * (DH + 1): qt * (DH + 1) + DH],
                        rr[:],
                        bv_b[:, h * DH:(h + 1) * DH],
                        op0=ALU.mult, op1=ALU.add,
                    )

            for m in range(NB):
                h0, h1 = 2 * m, 2 * m + 1
                if m > 0:
                    emit_K(m)
                    emit_Q(m)
                for s_ in range(KT16):
                    if s_ < KTP:
                        emit_scores_exp(h0, s_)
                    if m > 0:
                        emit_attnV_slot(h0 - 1, s_)
                if m > 0:
                    emit_norm(h0 - 1)
                for s_ in range(KT16):
                    if s_ < KTP:
                        emit_scores_exp(h1, s_)
                    emit_attnV_slot(h0, s_)
                emit_norm(h0)
                if m < NB - 2:
                    prefetch_w(m + 2)
                if m == 1:
                    # mid-phase prefetch of phase-C data (DMA slack here)
                    nc.sync.dma_start(
                        b1_c[:], b1_d.ap().rearrange("(b p) -> p b", p=P))
                    nc.sync.dma_start(
                        wp_sb[:].rearrange("p (k c) -> p k c", k=KB),
                        wp_d.ap().rearrange("(k p) c -> p k c", p=P))
                if m == 2:
                    nc.sync.dma_start(
                        xblk_sb[:].rearrange("p (q c) -> p q c", q=QT),
                        xblk_d.ap().rearrange("(q p) c -> p q c", p=P))
            # tail: head 15
            for s_ in range(KT16):
                emit_attnV_slot(H - 1, s_)
            emit_norm(H - 1)

        abp_stack.close()

        # ================= Phase C: outproj + LN1 + transpose =================
        cd = top.enter_context(tc.tile_pool(name="cd", bufs=1))
        xln1 = cd.tile([P, QT * D], FP32)           # LN1 out fp32, 16KB
        xln1T = cd.tile([P, KB * TPC], BF16)        # its transpose, 8KB

        with (
            tc.tile_pool(name="c_in", bufs=1) as cin,
            tc.tile_pool(name="c_scr", bufs=2) as pscr,
            tc.tile_pool(name="c_ps", bufs=4, space="PSUM") as ppp,
            tc.tile_pool(name="c_tp", bufs=2, space="PSUM") as tpp,
        ):
            # o^T: transpose o_nat head-pair blocks (PSUM banks free now)
            for pr in range(NB):
                tp = tpp.tile([P, QT * P], BF16, tag="tpo", name=f"tpo_{pr}")
                for qt in range(QT):
                    nc.tensor.transpose(
                        tp[:, qt * P:(qt + 1) * P],
                        o_nat[:, qt * D + pr * P: qt * D + (pr + 1) * P],
                        ident[:])
                for qt in range(QT):
                    nc.vector.tensor_copy(
                        oT[:, pr * TPC + qt * P: pr * TPC + (qt + 1) * P],
                        tp[:, qt * P:(qt + 1) * P])

            g1_b = cin.tile([P, D], FP32)
            nc.sync.dma_start(g1_b[:], g1_d.ap()[None, :].to_broadcast((P, D)))
            be1_b = cin.tile([P, D], FP32)
            nc.sync.dma_start(be1_b[:], be1_d.ap()[None, :].to_broadcast((P, D)))

            for qt in range(QT):
                pjs = []
                for nd in range(2):
                    pjt = ppp.tile([P, TPC], FP32, tag="projps",
                                   name=f"pj_{qt}_{nd}")
                    pjs.append(pjt)
                    for k in range(KB):
                        nc.tensor.matmul(
                            pjt[:],
                            lhsT=oT[:, k * TPC + qt * P: k * TPC + (qt + 1) * P],
                            rhs=wp_sb[:, k * D + nd * TPC: k * D + (nd + 1) * TPC],
                            start=(k == 0), stop=(k == KB - 1),
                        )
                y = pscr.tile([P, D], FP32, tag="y1")
                for nd in range(2):
                    nc.vector.tensor_add(
                        y[:, nd * TPC:(nd + 1) * TPC], pjs[nd][:],
                        xblk_sb[:, qt * D + nd * TPC: qt * D + (nd + 1) * TPC])
                _ln_chain(nc, pscr, y, xln1[:, qt * D:(qt + 1) * D],
                          g1_b, be1_b, eps_t, beta_eng=nc.gpsimd)
                # bf16 copy for the FFN1 transpose (on ACT, off the DVE chain)
                xb = cin.tile([P, D], BF16, name=f"xln1bf_{qt}")
                nc.scalar.copy(xb[:], xln1[:, qt * D:(qt + 1) * D])
                for half in range(2):
                    tp = tpp.tile([P, 4 * P], BF16, tag="tp2")
                    for j in range(4):
                        bd = half * 4 + j
                        nc.tensor.transpose(
                            tp[:, j * P:(j + 1) * P],
                            xb[:, bd * P:(bd + 1) * P],
                            ident[:])
                    for j in range(4):
                        bd = half * 4 + j
                        nc.vector.tensor_copy(
                            xln1T[:, bd * TPC + qt * P:
                                  bd * TPC + (qt + 1) * P],
                            tp[:, j * P:(j + 1) * P])

        # ================= Phase D: FFN =================
        with (
            tc.tile_pool(name="d_h", bufs=1) as fsb,
            tc.tile_pool(name="d_w", bufs=3) as fwp,
            tc.tile_pool(name="d_scr", bufs=2) as fscr,
        ):
            hT = fsb.tile([P, FB * TPC], BF16)    # relu(x@W1+b1)^T, 32KB
            xbp2 = fsb.tile([P, QT * D], FP32)    # xln1 + b2 (Pool), 16KB
            b2_b = fsb.tile([P, D], FP32)
            nc.sync.dma_start(b2_b[:], b2_d.ap()[None, :].to_broadcast((P, D)))
            g2_b = fsb.tile([P, D], FP32)
            nc.sync.dma_start(g2_b[:], g2_d.ap()[None, :].to_broadcast((P, D)))
            be2_b = fsb.tile([P, D], FP32)
            nc.sync.dma_start(be2_b[:], be2_d.ap()[None, :].to_broadcast((P, D)))

            MB1 = 4   # W1 mf tiles per batched DMA
            w1g_t = {}

            def w1_fetch(g):
                w1g_t[g] = fwp.tile([P, MB1 * KB * P], BF16, tag="w1g",
                                    name=f"w1g_{g}")
                nc.sync.dma_start(
                    w1g_t[g][:].rearrange("p (j c) -> p j c", j=MB1),
                    w1t_d.ap()[g * MB1 * P:(g + 1) * MB1 * P, :].rearrange(
                        "(j p) c -> p j c", p=P))

            w1_fetch(0)
            w1_fetch(1)

            with tc.tile_pool(name="d_f1_ps", bufs=8, space="PSUM") as fps:
                # first 8 mf: per-qt column groups, so FFN1 starts per-LN1
                phs = []
                for j in range(8):
                    ph = fps.tile([P, TPC], FP32, tag="fps", name=f"f1ps_{j}")
                    phs.append(ph)
                for qt in range(QT):
                    for j in range(8):
                        g, jj = divmod(j, MB1)
                        w1m = w1g_t[g]
                        for k in range(KB):
                            nc.tensor.matmul(
                                phs[j][:, qt * P:(qt + 1) * P],
                                lhsT=w1m[:, (jj * KB + k) * P:
                                         (jj * KB + k + 1) * P],
                                rhs=xln1T[:, k * TPC + qt * P:
                                          k * TPC + (qt + 1) * P],
                                start=(k == 0), stop=(k == KB - 1),
                            )
                    if qt in (0, 1):
                        w1_fetch(2 + qt)
                    # Pool folds b2 into xln1 while FFN1 runs
                    nc.gpsimd.tensor_add(
                        xbp2[:, qt * D:(qt + 1) * D],
                        xln1[:, qt * D:(qt + 1) * D], b2_b[:])
                for j in range(8):
                    nc.scalar.activation(
                        hT[:, j * TPC:(j + 1) * TPC], phs[j][:],
                        AF.Relu, bias=b1_c[:, j:j + 1])
                # remaining mf: plain 512-wide groups
                for mf in range(8, FB):
                    g, jj = divmod(mf, MB1)
                    if mf % MB1 == 0 and g + 2 <= (FB // MB1) - 1:
                        w1_fetch(g + 2)
                    w1m = w1g_t[g]
                    ph = fps.tile([P, TPC], FP32, tag="fps", name=f"f1ps_{mf}")
                    for k in range(KB):
                        nc.tensor.matmul(
                            ph[:],
                            lhsT=w1m[:, (jj * KB + k) * P:(jj * KB + k + 1) * P],
                            rhs=xln1T[:, k * TPC:(k + 1) * TPC],
                            start=(k == 0), stop=(k == KB - 1),
                        )
                    nc.scalar.activation(
                        hT[:, mf * TPC:(mf + 1) * TPC], ph[:], AF.Relu,
                        bias=b1_c[:, mf:mf + 1])

            MB2 = 4   # W2 k2 tiles per batched DMA
            with tc.tile_pool(name="d_f2_ps", bufs=8, space="PSUM") as fp2:
                for half in range(2):
                    qts = (2 * half, 2 * half + 1)
                    pj2 = {(qt, nd): fp2.tile([P, TPC], FP32, tag="f2ps",
                                              name=f"pj2_{qt}_{nd}")
                           for qt in qts for nd in range(2)}
                    w2g_t = {}

                    def w2_fetch(g, half=half, w2g_t=w2g_t):
                        w2g_t[g] = fwp.tile([P, MB2 * D], BF16, tag="w2g",
                                            name=f"w2g_{half}_{g}")
                        nc.sync.dma_start(
                            w2g_t[g][:].rearrange("p (j c) -> p j c", j=MB2),
                            w2_d.ap()[g * MB2 * P:(g + 1) * MB2 * P, :]
                            .rearrange("(j p) c -> p j c", p=P))

                    w2_fetch(0)
                    w2_fetch(1)
                    for k2 in range(FB):
                        g, jj = divmod(k2, MB2)
                        if k2 % MB2 == 0 and g + 2 <= (FB // MB2) - 1:
                            w2_fetch(g + 2)
                        w2k = w2g_t[g]
                        for qt in qts:
                            for nd in range(2):
                                nc.tensor.matmul(
                                    pj2[(qt, nd)][:],
                                    lhsT=hT[:, k2 * TPC + qt * P:
                                            k2 * TPC + (qt + 1) * P],
                                    rhs=w2k[:, jj * D + nd * TPC:
                                            jj * D + (nd + 1) * TPC],
                                    start=(k2 == 0), stop=(k2 == FB - 1),
                                )
                    for qt in qts:
                        y2 = fscr.tile([P, D], FP32, tag="y2")
                        for nd in range(2):
                            nc.vector.tensor_add(
                                y2[:, nd * TPC:(nd + 1) * TPC], pj2[(qt, nd)][:],
                                xbp2[:, qt * D + nd * TPC:
                                     qt * D + (nd + 1) * TPC])
                        yo = fscr.tile([P, D], FP32, tag="yo")
                        _ln_chain(nc, fscr, y2, yo[:], g2_b, be2_b, eps_t,
                                  beta_eng=nc.gpsimd)
                        nc.sync.dma_start(
                            out_d.ap()[qt * P:(qt + 1) * P, :], yo[:])


_PROG_CACHE: dict = {}


def _get_program(use_mask: bool) -> bass.Bass:
    if use_mask not in _PROG_CACHE:
        _PROG_CACHE[use_mask] = build_program(use_mask)
    return _PROG_CACHE[use_mask]


def make_in_maps(x, mask, Wq, bq, Wk, bk, Wv, bv, Wp, bp,
                 gamma1, beta1, W1, b1, W2, b2, gamma2, beta2):
    import ml_dtypes
    bf16 = ml_dtypes.bfloat16

    x = np.asarray(x, np.float32)
    mask = np.asarray(mask)
    use_mask = not bool(mask.all())

    def tiles_mk(W):
        # A[m, p, k, c] = W[k*128+p, m*128+c]  -> [M, 8*128] rows contiguous
        W = np.asarray(W, np.float32)
        nb = W.shape[1] // P
        A = W.reshape(KB, P, nb, P).transpose(2, 1, 0, 3).reshape(nb * P, KB * P)
        return np.ascontiguousarray(A.astype(bf16))

    common = {
        "wqt": tiles_mk(Wq),
        "wkt": tiles_mk(Wk),
        "wv": np.ascontiguousarray(np.asarray(Wv, np.float32).astype(bf16)),
        "wp": np.ascontiguousarray(np.asarray(Wp, np.float32).astype(bf16)),
        "w1t": tiles_mk(W1),
        "w2": np.ascontiguousarray(np.asarray(W2, np.float32).astype(bf16)),
        "bq": np.ascontiguousarray(bq, np.float32),
        "bk": np.ascontiguousarray(bk, np.float32),
        "bv": np.ascontiguousarray(np.asarray(bv, np.float32).astype(bf16)),
        "b1": np.ascontiguousarray(b1, np.float32),
        "b2": np.ascontiguousarray(b2, np.float32),
        "g1": np.ascontiguousarray(gamma1, np.float32),
        "be1": np.ascontiguousarray(beta1, np.float32),
        "g2": np.ascontiguousarray(gamma2, np.float32),
        "be2": np.ascontiguousarray(beta2, np.float32),
    }
    if use_mask:
        mbias = np.where(mask, np.float32(0.0), np.float32(-3e38)).astype(bf16)
    in_maps = []
    for c in range(N_CORES):
        b, j = divmod(c, 4)
        xb = x[b]
        m = dict(common)
        m["xT"] = np.ascontiguousarray(xb.T.astype(bf16))
        m["xTq"] = np.ascontiguousarray(
            xb[j * TPC:(j + 1) * TPC].T.astype(bf16))
        m["xblk"] = np.ascontiguousarray(
            (xb[j * TPC:(j + 1) * TPC] + np.asarray(bp, np.float32)
             ).astype(bf16))
        if use_mask:
            m["maskT"] = np.ascontiguousarray(mbias.T[:, j * TPC:(j + 1) * TPC])
        in_maps.append(m)
    return use_mask, in_maps


def assemble_output(results) -> np.ndarray:
    out = np.empty((2, S, D), np.float32)
    for c in range(N_CORES):
        b, j = divmod(c, 4)
        out[b, j * TPC:(j + 1) * TPC] = results[c]["out"]
    return out


def kernel(**inputs) -> np.ndarray:
    use_mask, in_maps = make_in_maps(**inputs)
    nc = _get_program(use_mask)
    res = run_bass_kernel_spmd(nc, in_maps, list(range(N_CORES)))
    return assemble_output(res.results)
